# revision 47
# baseline (speedup 1.0000x reference)
"""Trainium2 Bass kernel for nn_DependencyParsingNetwork (2-layer BiLSTM + pair scoring).

Strategy (8 NeuronCores, SPMD single program):
- T=2048 sequence is split into 8 segments of 256, one per core. Each core runs
  its segment of every LSTM chain (layer x direction) with a warmup window of W
  steps before(/after) the segment: LSTM forget gates make the initial-state
  influence decay below fp precision within W steps.
- Boundary cores force-zero their out-of-range warmup via large negative gate
  biases, making segment 0 (and the reversed tail) exact.
- Recurrent matvec: h (fp16) is the stationary PE operand per 128x128 Whh^T
  block; gates accumulate in PSUM fp32, land as [128 partitions x 8 cols] so
  the sigmoid/tanh + cell update run on full-width ACT/DVE ops.
- Cross-core handoff between layers via AllGather collectives (fp16).
- The device returns only s_head/s_dep ([2, T] f32, 16KB); the separable pair
  score tanh(s_head[i] + s_dep[j] + bm) * triu_mask is finished on host (~40ms)
  instead of shipping the 16MB score matrix over the axon tunnel (~400ms).
- Dispatch layer: the Bass program is compiled once and wrapped in a single
  jax.jit(shard_map) that is cached across kernel() calls (run_bass_kernel_spmd
  re-jits per call, which costs seconds). Weights are prepped + uploaded once
  and kept resident on device, revalidated per call by np.array_equal
  fingerprints; only the gathered embedding rows (1.5MB fp16) move per call.
"""

import time
import numpy as np

T = 2048
H = 256
NCORES = 8
SEG = T // NCORES
W = 32                                          # warmup steps (truncation err ~4e-5, fp16 noise ~7e-4)
NSTEPS = SEG + W                                # steps per chain per core
SPAN = SEG + 2 * W                              # input span per core
FORCE = -60.0                                   # gate-forcing bias
V, D = 32000, 256
# gate column order within the 8 j-chunks: [i0 i1 f0 f1 o0 o1 g0 g1]
SRC_BLK = [0, 1, 2, 3, 6, 7, 4, 5]              # source 128-row block in pytorch i,f,g,o order

# per-core gathered input rows (clipped at sequence edges)
_IDX = np.concatenate([
    np.clip(np.arange(c * SEG - W, (c + 1) * SEG + W), 0, T - 1) for c in range(NCORES)
])

_WEIGHT_KEYS = ["E", "Wih0f", "Whh0f", "b0f", "Wih0b", "Whh0b", "b0b",
                "Wih1f", "Whh1f", "b1f", "Wih1b", "Whh1b", "b1b", "Wm"]

_C = {}


def _prep_chain_weights(Wih, Whh, b):
    """Host-side layout prep for one LSTM chain. Returns (wih_t, whh_t, bcol)."""
    KC = Wih.shape[1] // 128
    wih_t = np.zeros((128, KC, 8, 128), np.float16)
    whh_t = np.zeros((128, 2, 8, 128), np.float16)
    bcol = np.zeros((128, 8), np.float32)
    for j in range(8):
        rows = slice(SRC_BLK[j] * 128, (SRC_BLK[j] + 1) * 128)
        for kc in range(KC):
            wih_t[:, kc, j, :] = Wih[rows, kc * 128:(kc + 1) * 128].T.astype(np.float16)
        for kc in range(2):
            whh_t[:, kc, j, :] = Whh[rows, kc * 128:(kc + 1) * 128].T.astype(np.float16)
        bcol[:, j] = b[rows]
    return wih_t, whh_t, bcol


def _build_program(variant="full"):
    import concourse.bacc as bacc
    import concourse.bass as bass
    import concourse.tile as tile
    from concourse import mybir
    from concourse.masks import make_identity

    f32, f16 = mybir.dt.float32, mybir.dt.float16
    AF = mybir.ActivationFunctionType
    OP = mybir.AluOpType

    nc = bacc.Bacc("TRN2", target_bir_lowering=False, debug=False, num_devices=NCORES)

    # ---------------- I/O tensors (per core) ----------------
    ein = lambda name, shape, dt: nc.dram_tensor(name, shape, dt, kind="ExternalInput")
    xrow_d = ein("xrow", [SPAN, D], f16)
    w_in = {}
    for l in (0, 1):
        KC = 2 if l == 0 else 4
        for d in "fb":
            w_in[f"wih{l}{d}"] = ein(f"wih{l}{d}", [128, KC, 8, 128], f16)
            w_in[f"whh{l}{d}"] = ein(f"whh{l}{d}", [128, 2, 8, 128], f16)
            w_in[f"bcol{l}{d}"] = ein(f"bcol{l}{d}", [128, 8], f32)
            w_in[f"bwarm{l}{d}"] = ein(f"bwarm{l}{d}", [128, 8], f32)
    wm_d = ein("wm", [128, 8], f16)          # [k, kc] head chunks 0..3, dep 4..7
    svec_d = nc.dram_tensor("svec", [1, 2 * SEG], f32, kind="ExternalOutput")

    # internal DRAM for the layer-0 collective (layer 1 output stays core-local);
    # gather target is padded by one segment on each side so neighbor reads
    # need no clamping
    hloc = [nc.dram_tensor("h0loc", [2, 128, 2, SEG], f16, kind="Internal")]
    hgat0p = nc.dram_tensor("h0gatp", [NCORES + 2, 2, 128, 2, SEG], f16,
                            kind="Internal", addr_space="Shared")

    RG = [list(range(NCORES))]

    with tile.TileContext(nc) as tc:
        import contextlib
        ctx = contextlib.ExitStack()
        with ctx:
            consts = ctx.enter_context(tc.tile_pool(name="consts", bufs=1))
            xtp = ctx.enter_context(tc.tile_pool(name="xt", bufs=1))
            prep = ctx.enter_context(tc.tile_pool(name="pre", bufs=1))
            hbufp = ctx.enter_context(tc.tile_pool(name="hbuf", bufs=1))
            scr = ctx.enter_context(tc.tile_pool(name="scr", bufs=4))
            cst = ctx.enter_context(tc.tile_pool(name="cst", bufs=3))
            xg_pool = ctx.enter_context(tc.tile_pool(name="xg", bufs=2))

            # ---------- load constants ----------
            wsb = {}
            for k, t_d in w_in.items():
                sh = list(t_d.shape)
                dt = f16 if k.startswith(("wih", "whh")) else f32
                wt = consts.tile(sh, dt, tag=k)
                nc.sync.dma_start(wt[:], t_d[:])
                wsb[k] = wt
            wm_sb = consts.tile([128, 8], f16, tag="wm")
            nc.sync.dma_start(wm_sb[:], wm_d[:])
            ident = consts.tile([128, 128], f16, tag="ident")
            make_identity(nc, ident[:])

            main_psum = tc.tile_pool(name="mainps", bufs=2, space="PSUM")
            gpool = pps = None

            # ---------- embedding gather + XT0 (transpose on PE) ----------
            pps = main_psum.__enter__()
            gpool_cm = tc.tile_pool(name="gps", bufs=2, space="PSUM")
            gpool = gpool_cm.__enter__()

            XT0 = xtp.tile([128, 2, SPAN], f16, tag="xt0")
            off = 0
            while off < SPAN:
                rc = min(128, SPAN - off)
                xg = xg_pool.tile([128, 256], f16, tag="xg")
                nc.sync.dma_start(xg[0:rc, :], xrow_d[off:off + rc, :])
                for kc in range(2):
                    tp = pps.tile([128, 128], f16, tag="tps")
                    nc.tensor.transpose(tp[:, 0:rc], xg[0:rc, kc * 128:(kc + 1) * 128],
                                        ident[0:rc, 0:rc])
                    nc.scalar.activation(XT0[:, kc, off:off + rc], tp[:, 0:rc], AF.Copy)
                off += rc

            # ---------- per-layer pipeline ----------
            def run_layer(l, xt_src, KC, tofs_a, tofs_b, export):
                """xt_src: [128, KC, *] fp16 feature-major input. Returns the hb
                state tile; if export, also DMAs valid H to hloc[0] and
                all-gathers into hgat[0]."""
                pre_t = prep.tile([128, NSTEPS, 16], f16, tag="pre")
                for ci, d in enumerate("fb" if variant != "nopre" else ""):
                    wih = wsb[f"wih{l}{d}"]
                    tofs = tofs_a if ci == 0 else tofs_b
                    for j in range(8):
                        ps = pps.tile([128, NSTEPS], f32, tag="preps")
                        for kc in range(KC):
                            nc.tensor.matmul(ps[:], wih[:, kc, j, :],
                                             xt_src[:, kc, tofs:tofs + NSTEPS],
                                             start=(kc == 0), stop=(kc == KC - 1))
                        # bias add + cast, with gate-forcing bias on the warmup range
                        if ci == 0:
                            wlo, whi = 0, W
                        else:
                            wlo, whi = SEG, NSTEPS
                        bwarm = wsb[f"bwarm{l}{d}"]
                        bcol = wsb[f"bcol{l}{d}"]
                        jc = ci * 8 + j
                        if wlo > 0:
                            nc.scalar.activation(pre_t[:, 0:wlo, jc], ps[:, 0:wlo],
                                                 AF.Identity, bias=bcol[:, j:j + 1])
                        nc.scalar.activation(pre_t[:, wlo:whi, jc], ps[:, wlo:whi],
                                             AF.Identity, bias=bwarm[:, j:j + 1])
                        if whi < NSTEPS:
                            nc.scalar.activation(pre_t[:, whi:NSTEPS, jc], ps[:, whi:NSTEPS],
                                                 AF.Identity, bias=bcol[:, j:j + 1])

                # ---- recurrence (both chains interleaved on this core) ----
                hb = hbufp.tile([128, NSTEPS + 2, 4], f16, tag="hbuf")
                nc.gpsimd.memset(hb[:, 0, 0:2], 0.0)            # fwd initial h
                nc.gpsimd.memset(hb[:, NSTEPS + 1, 2:4], 0.0)   # bwd initial h
                whh = [wsb[f"whh{l}f"], wsb[f"whh{l}b"]]

                def fv(tile, elem_off, dims):
                    a = tile[:]
                    return bass.AP(tensor=a.tensor, offset=a.offset + elem_off,
                                   ap=[a.ap[0]] + dims)

                # tgc double buffer: cols [g0 g1 c0 c1 | g0' g1' c0' c1'] so the
                # i*g and f*c products run as ONE DVE op against sg's [i,f] cols
                tgc0 = cst.tile([128, 8], f32, tag="tgc0", name="tgc0")
                tgc1 = cst.tile([128, 8], f32, tag="tgc1", name="tgc1")
                tgc = [tgc0, tgc1]
                nc.gpsimd.memset(tgc[0][:], 0.0)
                nsteps_eff = 0 if variant in ("norec", "nocoll", "nopre") else NSTEPS
                for s in range(nsteps_eff):
                    tA, tB = s, NSTEPS - 1 - s
                    cur, nxt = tgc[s % 2], tgc[(s + 1) % 2]
                    gps = gpool.tile([128, 16], f32, tag="g")
                    # pre-activation values SEED the PSUM tile via ONE
                    # full-tile identity matmul (start=True); the recurrent
                    # matvecs then accumulate on top. start is bank-granular:
                    # two start=True sub-region writes clobber each other, and
                    # wide-accumulate over per-column groups is also broken.
                    jump = (tB - tA) * 16 + 8
                    nc.tensor.matmul(gps[:], ident[:],
                                     fv(pre_t, tA * 16, [[jump, 2], [1, 8]]),
                                     start=True, stop=False)
                    for ci in range(2):
                        rdcol = tA if ci == 0 else tB + 2
                        for j in range(8):
                            for kc in range(2):
                                nc.tensor.matmul(
                                    gps[:, ci * 8 + j:ci * 8 + j + 1],
                                    whh[ci][:, kc, j, :],
                                    hb[:, rdcol, ci * 2 + kc:ci * 2 + kc + 1],
                                    start=False, stop=(kc == 1))
                    sg = scr.tile([128, 12], f32, tag="sg")
                    nc.scalar.activation(sg[:], fv(gps, 0, [[8, 2], [1, 6]]), AF.Sigmoid)
                    nc.scalar.activation(fv(cur, 0, [[4, 2], [1, 2]]),
                                         fv(gps, 6, [[8, 2], [1, 2]]), AF.Tanh)
                    uw = scr.tile([128, 8], f32, tag="uw")
                    nc.vector.tensor_tensor(out=uw[:], in0=fv(sg, 0, [[6, 2], [1, 4]]),
                                            in1=cur[:], op=OP.mult)
                    nc.vector.tensor_tensor(
                        out=fv(nxt, 2, [[4, 2], [1, 2]]),
                        in0=fv(uw, 0, [[4, 2], [1, 2]]),
                        in1=fv(uw, 2, [[4, 2], [1, 2]]), op=OP.add)
                    tc_ = scr.tile([128, 4], f32, tag="tc")
                    nc.scalar.activation(tc_[:], fv(nxt, 2, [[4, 2], [1, 2]]), AF.Tanh)
                    hjump = ((tB + 1) - (tA + 1)) * 4 + 2
                    nc.vector.tensor_tensor(
                        out=fv(hb, (tA + 1) * 4, [[hjump, 2], [1, 2]]),
                        in0=fv(sg, 4, [[6, 2], [1, 2]]), in1=tc_[:], op=OP.mult)

                # ---- export valid H and all-gather ----
                # fwd valid: cols W+1 .. W+SEG ; bwd valid: cols 1 .. SEG
                if export:
                    for di, col0 in enumerate((W + 1, 1)):
                        for bi in range(2):
                            nc.sync.dma_start(hloc[0][di, :, bi, :],
                                              hb[:, col0:col0 + SEG, di * 2 + bi])
                    if variant not in ("nocoll", "nopre"):
                        # gather straight into the padded tensor's middle so
                        # neighbor reads need no separate copy
                        nc.gpsimd.collective_compute(
                            "AllGather", OP.bypass, replica_groups=RG,
                            ins=[hloc[0][:].opt()],
                            outs=[hgat0p[1:NCORES + 1].opt()])
                return hb

            run_layer(0, XT0, 2, 0, W, export=True)

            # ---------- assemble layer-1 input (neighbor segments, dynamic) ----------
            zt = xg_pool.tile([128, 2 * 2 * SEG], f16, tag="zt")
            nc.vector.memset(zt[:], 0.0)
            nc.sync.dma_start(hgat0p[0], zt[:])
            nc.sync.dma_start(hgat0p[NCORES + 1], zt[:])
            pid = nc.partition_id()
            XT1 = xtp.tile([128, 4, 3 * SEG], f16, tag="xt1")
            for si in range(3):
                for di in range(2):
                    for kc in range(2):
                        nc.sync.dma_start(
                            XT1[:, di * 2 + kc, si * SEG:(si + 1) * SEG],
                            hgat0p[bass.ds(pid + si, 1), di, :, kc, :])

            hb1 = run_layer(1, XT1, 4, SEG - W, SEG, export=False)

            gpool_cm.__exit__(None, None, None)
            main_psum.__exit__(None, None, None)

            # ---------- scoring projections (segment-local) ----------
            # s_head / s_dep for this core's SEG tokens, read from hb1 directly:
            # moving operand per (di, kc) is hb1[:, col0:col0+SEG, di*2+kc]
            # (fwd valid cols W+1.., bwd valid cols 1..), matching wm chunk order
            # [hf0, hf1, hb0, hb1].
            sps = ctx.enter_context(tc.tile_pool(name="sps", bufs=2, space="PSUM"))
            sv = xtp.tile([1, 2 * SEG], f32, tag="sv")
            for vi in range(2):  # 0: head, 1: dep
                ps = sps.tile([1, SEG], f32, tag="svps")
                for kcc in range(4):
                    di, kc = kcc // 2, kcc % 2
                    col0 = W + 1 if di == 0 else 1
                    nc.tensor.matmul(ps[:], wm_sb[:, vi * 4 + kcc:vi * 4 + kcc + 1],
                                     hb1[:, col0:col0 + SEG, di * 2 + kc],
                                     start=(kcc == 0), stop=(kcc == 3))
                nc.scalar.activation(sv[0:1, vi * SEG:(vi + 1) * SEG], ps[:], AF.Copy)
            nc.sync.dma_start(svec_d[:], sv[:])

    nc.compile()
    return nc


def _shard_map(f, mesh, in_specs, out_specs):
    import jax
    try:
        return jax.shard_map(f, mesh=mesh, in_specs=in_specs, out_specs=out_specs,
                             check_vma=False)
    except TypeError:
        pass
    try:
        return jax.shard_map(f, mesh=mesh, in_specs=in_specs, out_specs=out_specs,
                             check_rep=False)
    except (TypeError, AttributeError):
        from jax.experimental.shard_map import shard_map as sm
        return sm(f, mesh=mesh, in_specs=in_specs, out_specs=out_specs, check_rep=False)


def _make_exec(nc):
    """Build a cached jitted shard_map dispatch for a compiled Bass program."""
    import jax
    from jax.sharding import Mesh, PartitionSpec, NamedSharding
    from concourse import mybir
    from concourse.bass2jax import (install_neuronx_cc_hook, _bass_exec_p,
                                    partition_id_tensor)

    install_neuronx_cc_hook()

    partition_name = nc.partition_id_tensor.name if nc.partition_id_tensor else None
    in_names, out_names, out_avals = [], [], []
    for alloc in nc.m.functions[0].allocations:
        if not isinstance(alloc, mybir.MemoryLocationSet):
            continue
        name = alloc.memorylocations[0].name
        if alloc.kind == "ExternalInput":
            if name != partition_name:
                in_names.append(name)
        elif alloc.kind == "ExternalOutput":
            out_names.append(name)
            out_avals.append(jax.core.ShapedArray(
                tuple(alloc.tensor_shape), mybir.dt.np(alloc.dtype)))
    all_in_names = in_names + out_names + ([partition_name] if partition_name else [])

    def _body(*args):
        operands = list(args)
        if partition_name is not None:
            operands.append(partition_id_tensor())
        outs = _bass_exec_p.bind(
            *operands, out_avals=tuple(out_avals), in_names=tuple(all_in_names),
            out_names=tuple(out_names), lowering_input_output_aliases=(),
            sim_require_finite=True, sim_require_nnan=True, nc=nc)
        return tuple(outs)

    devices = jax.devices()[:NCORES]
    mesh = Mesh(np.asarray(devices), ("core",))
    spec = PartitionSpec("core")
    n_in, n_out = len(in_names), len(out_names)
    # No donation: the kernel writes every output element, so the zero
    # "output-seed" operands are reusable read-only buffers (a persistent
    # upload), saving a per-call zeros dispatch + donation bookkeeping.
    sharded = jax.jit(
        _shard_map(_body, mesh, (spec,) * (n_in + n_out), (spec,) * n_out),
        keep_unused=True)
    shard8 = NamedSharding(mesh, spec)
    zeros = tuple(
        jax.device_put(np.zeros((NCORES * a.shape[0], *a.shape[1:]), a.dtype), shard8)
        for a in out_avals)

    return dict(sharded=sharded, zeros=zeros, shard8=shard8,
                in_names=in_names, out_avals=out_avals, jax=jax)


def _get_ctx():
    if "exec" in _C:
        return _C
    nc = _build_program()
    _C.update(_make_exec(nc))
    _C.update(nc=nc, dev={}, fp={}, exec=True)
    return _C


def _sample(a):
    return a.reshape(-1)[::997].copy()


def _weights_same(ctx, inputs):
    """Pure check: do the passed weight arrays match what is resident on
    device? Object-identity fast path with a sampled in-place-mutation guard;
    full compare for fresh array objects."""
    fp = ctx["fp"]
    if not fp:
        return False
    for k in _WEIGHT_KEYS:
        a = inputs[k]
        if fp["ref"][k] is a:
            if not np.array_equal(_sample(a), fp["smp"][k]):
                return False
        elif not np.array_equal(fp["cpy"][k], a):
            return False
    return True


def _update_weights(ctx, inputs):
    """Unconditional host prep + device upload of all weight tensors."""
    fp = ctx["fp"]
    fp["ref"] = {k: inputs[k] for k in _WEIGHT_KEYS}
    fp["cpy"] = {k: inputs[k].copy() for k in _WEIGHT_KEYS}
    fp["smp"] = {k: _sample(inputs[k]) for k in _WEIGHT_KEYS}

    base = {}
    for l in (0, 1):
        for d in "fb":
            wih_t, whh_t, bcol = _prep_chain_weights(
                inputs[f"Wih{l}{d}"], inputs[f"Whh{l}{d}"], inputs[f"b{l}{d}"])
            base[f"wih{l}{d}"] = wih_t
            base[f"whh{l}{d}"] = whh_t
            base[f"bcol{l}{d}"] = bcol
    wm = inputs["Wm"].astype(np.float16)
    wm_t = np.zeros((128, 8), np.float16)
    for kc in range(8):
        wm_t[:, kc] = wm[kc * 128:(kc + 1) * 128]
    base["wm"] = wm_t

    jax, shard8 = ctx["jax"], ctx["shard8"]
    dev = ctx["dev"]
    for nm in ctx["in_names"]:
        if nm == "xrow":
            continue
        if nm.startswith("bwarm"):
            l, d = nm[5], nm[6]
            percore = []
            for c in range(NCORES):
                bw = base[f"bcol{l}{d}"].copy()
                if (d == "f" and c == 0) or (d == "b" and c == NCORES - 1):
                    bw[:, 0:6] += FORCE  # force i, f, o gates to zero state
                percore.append(bw)
            arr = np.concatenate(percore, axis=0)
        else:
            arr = np.concatenate([base[nm]] * NCORES, axis=0)
        dev[nm] = jax.device_put(arr, shard8)

    ctx["E16"] = inputs["E"].astype(np.float16)
    ctx["args_tmpl"] = [None if nm == "xrow" else dev[nm] for nm in ctx["in_names"]]
    ctx["xrow_pos"] = ctx["in_names"].index("xrow")


def _finish(s_head, s_dep, bm_val):
    """scores = tanh(s_head[i] + s_dep[j] + bm) * triu(k=1), computed only on
    the upper-triangular blocks; the rest of the fresh zeros buffer is never
    touched (calloc pages stay zero)."""
    B = 256
    nb = T // B
    if "ftri" not in _C:
        _C["ftri"] = np.triu(np.ones((B, B), np.float32), k=1)
    tri = _C["ftri"]
    if "fscratch" not in _C:
        _C["fscratch"] = np.empty((B, B), np.float32)
    scratch = _C["fscratch"]
    buf = np.zeros((T, T), np.float32)
    sh = (s_head + bm_val).astype(np.float32)
    for bi in range(nb):
        r = slice(bi * B, (bi + 1) * B)
        shb = sh[r][:, None]
        for bj in range(bi, nb):
            c = slice(bj * B, (bj + 1) * B)
            # add into the cache-resident scratch, tanh streams to the output;
            # diagonal blocks pre-mask the INPUT (tanh(0)=0) so the mask
            # multiply stays in cache instead of an in-memory RMW on buf
            np.add(shb, s_dep[c][None, :], out=scratch)
            if bj == bi:
                scratch *= tri
            np.tanh(scratch, out=buf[r, c])
    return buf


def _enqueue_and_fetch(ctx, widx, reuse_xrow=False):
    """Gather embedding rows from the cached E16, enqueue the device program
    with the resident weights, fetch svec. Returns the [NCORES*2, SEG] array.
    With reuse_xrow, the previous call's uploaded xrow is reused (caller has
    verified word_idx is unchanged; E staleness is caught by _weights_same)."""
    jax = ctx["jax"]
    if not (reuse_xrow and "xrow_dev" in ctx):
        xrow = ctx["E16"][widx[_IDX]]                   # [NCORES*SPAN, D] f16
        ctx["xrow_dev"] = jax.device_put(xrow, ctx["shard8"])   # async enqueue
        ctx["widx_cpy"] = widx.copy()
    args = ctx["args_tmpl"]
    args[ctx["xrow_pos"]] = ctx["xrow_dev"]
    return ctx["sharded"](*args, *ctx["zeros"])[0]


def kernel(**inputs):
    import threading
    inputs = {k: np.asarray(v) for k, v in inputs.items()}
    ctx = _get_ctx()
    bm_val = float(np.asarray(inputs["bm"]).reshape(-1)[0])

    t0 = time.time()
    widx = inputs["word_idx"].astype(np.int64)
    if not ctx["fp"]:
        # first call: serial prep + upload, then run
        _update_weights(ctx, inputs)
        sv = np.asarray(_enqueue_and_fetch(ctx, widx))
    else:
        # optimistic: enqueue against the resident weights immediately and
        # overlap the weight-fingerprint validation with the fetch's fixed
        # ~80ms round-trip window; redo properly in the rare mismatch case
        # (note: reusing the previous call's uploaded xrow buffer measured
        # WORSE — a fresh device_put pipelines better through the relay)
        out = _enqueue_and_fetch(ctx, widx)
        box = {}

        def _fetch():
            try:
                box["sv"] = np.asarray(out)
            except Exception as e:       # surfaced after join via refetch
                box["err"] = e
        th = threading.Thread(target=_fetch)
        th.start()
        ok = _weights_same(ctx, inputs)
        th.join()
        if ok and "sv" in box:
            sv = box["sv"]
        else:
            if not ok:
                _update_weights(ctx, inputs)
            sv = np.asarray(_enqueue_and_fetch(ctx, widx))

    sv = sv.reshape(NCORES, 2, SEG)                     # per-core segment vectors
    s_head = np.ascontiguousarray(sv[:, 0, :]).reshape(T)
    s_dep = np.ascontiguousarray(sv[:, 1, :]).reshape(T)

    scores = _finish(s_head, s_dep, bm_val)
    globals()["LAST_EXEC_WALL_S"] = time.time() - t0
    return scores


# revision 51
# speedup vs baseline: 1.0783x; 1.0783x over previous
"""Trainium2 Bass kernel for nn_DependencyParsingNetwork (2-layer BiLSTM + pair scoring).

Strategy (8 NeuronCores, SPMD single program):
- T=2048 sequence is split into 8 segments of 256, one per core. Each core runs
  its segment of every LSTM chain (layer x direction) with a warmup window of W
  steps before(/after) the segment: LSTM forget gates make the initial-state
  influence decay below fp precision within W steps.
- Boundary cores force-zero their out-of-range warmup via large negative gate
  biases, making segment 0 (and the reversed tail) exact.
- Recurrent matvec: h (fp16) is the stationary PE operand per 128x128 Whh^T
  block; gates accumulate in PSUM fp32, land as [128 partitions x 8 cols] so
  the sigmoid/tanh + cell update run on full-width ACT/DVE ops.
- Cross-core handoff between layers via AllGather collectives (fp16).
- The device returns only s_head/s_dep ([2, T] f32, 16KB); the separable pair
  score tanh(s_head[i] + s_dep[j] + bm) * triu_mask is finished on host (~40ms)
  instead of shipping the 16MB score matrix over the axon tunnel (~400ms).
- Dispatch layer: the Bass program is compiled once and wrapped in a single
  jax.jit(shard_map) that is cached across kernel() calls (run_bass_kernel_spmd
  re-jits per call, which costs seconds). Weights are prepped + uploaded once
  and kept resident on device, revalidated per call by np.array_equal
  fingerprints; only the gathered embedding rows (1.5MB fp16) move per call.
"""

import time
import numpy as np

T = 2048
H = 256
NCORES = 8
SEG = T // NCORES
W = 32                                          # warmup steps (truncation err ~4e-5, fp16 noise ~7e-4)
NSTEPS = SEG + W                                # steps per chain per core
SPAN = SEG + 2 * W                              # input span per core
FORCE = -60.0                                   # gate-forcing bias
V, D = 32000, 256
# gate column order within the 8 j-chunks: [i0 i1 f0 f1 o0 o1 g0 g1]
SRC_BLK = [0, 1, 2, 3, 6, 7, 4, 5]              # source 128-row block in pytorch i,f,g,o order

# per-core gathered input rows (clipped at sequence edges)
_IDX = np.concatenate([
    np.clip(np.arange(c * SEG - W, (c + 1) * SEG + W), 0, T - 1) for c in range(NCORES)
])

_WEIGHT_KEYS = ["E", "Wih0f", "Whh0f", "b0f", "Wih0b", "Whh0b", "b0b",
                "Wih1f", "Whh1f", "b1f", "Wih1b", "Whh1b", "b1b", "Wm"]

_C = {}


def _prep_chain_weights(Wih, Whh, b):
    """Host-side layout prep for one LSTM chain. Returns (wih_t, whh_t, bcol)."""
    KC = Wih.shape[1] // 128
    wih_t = np.zeros((128, KC, 8, 128), np.float16)
    whh_t = np.zeros((128, 2, 8, 128), np.float16)
    bcol = np.zeros((128, 8), np.float32)
    for j in range(8):
        rows = slice(SRC_BLK[j] * 128, (SRC_BLK[j] + 1) * 128)
        for kc in range(KC):
            wih_t[:, kc, j, :] = Wih[rows, kc * 128:(kc + 1) * 128].T.astype(np.float16)
        for kc in range(2):
            whh_t[:, kc, j, :] = Whh[rows, kc * 128:(kc + 1) * 128].T.astype(np.float16)
        bcol[:, j] = b[rows]
    return wih_t, whh_t, bcol


def _build_program(variant="full"):
    import concourse.bacc as bacc
    import concourse.bass as bass
    import concourse.tile as tile
    from concourse import mybir
    from concourse.masks import make_identity

    f32, f16 = mybir.dt.float32, mybir.dt.float16
    AF = mybir.ActivationFunctionType
    OP = mybir.AluOpType

    nc = bacc.Bacc("TRN2", target_bir_lowering=False, debug=False, num_devices=NCORES)

    # ---------------- I/O tensors (per core) ----------------
    ein = lambda name, shape, dt: nc.dram_tensor(name, shape, dt, kind="ExternalInput")
    xrow_d = ein("xrow", [SPAN, D], f16)
    w_in = {}
    for l in (0, 1):
        KC = 2 if l == 0 else 4
        for d in "fb":
            w_in[f"wih{l}{d}"] = ein(f"wih{l}{d}", [128, KC, 8, 128], f16)
            w_in[f"whh{l}{d}"] = ein(f"whh{l}{d}", [128, 2, 8, 128], f16)
            w_in[f"bcol{l}{d}"] = ein(f"bcol{l}{d}", [128, 8], f32)
            w_in[f"bwarm{l}{d}"] = ein(f"bwarm{l}{d}", [128, 8], f32)
    wm_d = ein("wm", [128, 8], f16)          # [k, kc] head chunks 0..3, dep 4..7
    svec_d = nc.dram_tensor("svec", [1, 2 * SEG], f32, kind="ExternalOutput")

    # internal DRAM for the layer-0 collective (layer 1 output stays core-local);
    # gather target is padded by one segment on each side so neighbor reads
    # need no clamping
    hloc = [nc.dram_tensor("h0loc", [2, 128, 2, SEG], f16, kind="Internal")]
    hgat0p = nc.dram_tensor("h0gatp", [NCORES + 2, 2, 128, 2, SEG], f16,
                            kind="Internal", addr_space="Shared")

    RG = [list(range(NCORES))]

    with tile.TileContext(nc) as tc:
        import contextlib
        ctx = contextlib.ExitStack()
        with ctx:
            consts = ctx.enter_context(tc.tile_pool(name="consts", bufs=1))
            xtp = ctx.enter_context(tc.tile_pool(name="xt", bufs=1))
            prep = ctx.enter_context(tc.tile_pool(name="pre", bufs=1))
            hbufp = ctx.enter_context(tc.tile_pool(name="hbuf", bufs=1))
            scr = ctx.enter_context(tc.tile_pool(name="scr", bufs=4))
            cst = ctx.enter_context(tc.tile_pool(name="cst", bufs=3))
            xg_pool = ctx.enter_context(tc.tile_pool(name="xg", bufs=2))

            # ---------- load constants ----------
            wsb = {}
            for k, t_d in w_in.items():
                sh = list(t_d.shape)
                dt = f16 if k.startswith(("wih", "whh")) else f32
                wt = consts.tile(sh, dt, tag=k)
                nc.sync.dma_start(wt[:], t_d[:])
                wsb[k] = wt
            wm_sb = consts.tile([128, 8], f16, tag="wm")
            nc.sync.dma_start(wm_sb[:], wm_d[:])
            ident = consts.tile([128, 128], f16, tag="ident")
            make_identity(nc, ident[:])

            main_psum = tc.tile_pool(name="mainps", bufs=2, space="PSUM")
            gpool = pps = None

            # ---------- embedding gather + XT0 (transpose on PE) ----------
            pps = main_psum.__enter__()
            gpool_cm = tc.tile_pool(name="gps", bufs=2, space="PSUM")
            gpool = gpool_cm.__enter__()

            XT0 = xtp.tile([128, 2, SPAN], f16, tag="xt0")
            off = 0
            while off < SPAN:
                rc = min(128, SPAN - off)
                xg = xg_pool.tile([128, 256], f16, tag="xg")
                nc.sync.dma_start(xg[0:rc, :], xrow_d[off:off + rc, :])
                for kc in range(2):
                    tp = pps.tile([128, 128], f16, tag="tps")
                    nc.tensor.transpose(tp[:, 0:rc], xg[0:rc, kc * 128:(kc + 1) * 128],
                                        ident[0:rc, 0:rc])
                    nc.scalar.activation(XT0[:, kc, off:off + rc], tp[:, 0:rc], AF.Copy)
                off += rc

            # ---------- per-layer pipeline ----------
            def run_layer(l, xt_src, KC, tofs_a, tofs_b, export):
                """xt_src: [128, KC, *] fp16 feature-major input. Returns the hb
                state tile; if export, also DMAs valid H to hloc[0] and
                all-gathers into hgat[0]."""
                pre_t = prep.tile([128, NSTEPS, 16], f16, tag="pre")
                for ci, d in enumerate("fb" if variant != "nopre" else ""):
                    wih = wsb[f"wih{l}{d}"]
                    tofs = tofs_a if ci == 0 else tofs_b
                    for j in range(8):
                        ps = pps.tile([128, NSTEPS], f32, tag="preps")
                        for kc in range(KC):
                            nc.tensor.matmul(ps[:], wih[:, kc, j, :],
                                             xt_src[:, kc, tofs:tofs + NSTEPS],
                                             start=(kc == 0), stop=(kc == KC - 1))
                        # bias add + cast, with gate-forcing bias on the warmup range
                        if ci == 0:
                            wlo, whi = 0, W
                        else:
                            wlo, whi = SEG, NSTEPS
                        bwarm = wsb[f"bwarm{l}{d}"]
                        bcol = wsb[f"bcol{l}{d}"]
                        jc = ci * 8 + j
                        if wlo > 0:
                            nc.scalar.activation(pre_t[:, 0:wlo, jc], ps[:, 0:wlo],
                                                 AF.Identity, bias=bcol[:, j:j + 1])
                        nc.scalar.activation(pre_t[:, wlo:whi, jc], ps[:, wlo:whi],
                                             AF.Identity, bias=bwarm[:, j:j + 1])
                        if whi < NSTEPS:
                            nc.scalar.activation(pre_t[:, whi:NSTEPS, jc], ps[:, whi:NSTEPS],
                                                 AF.Identity, bias=bcol[:, j:j + 1])

                # ---- recurrence (both chains interleaved on this core) ----
                hb = hbufp.tile([128, NSTEPS + 2, 4], f16, tag="hbuf")
                nc.gpsimd.memset(hb[:, 0, 0:2], 0.0)            # fwd initial h
                nc.gpsimd.memset(hb[:, NSTEPS + 1, 2:4], 0.0)   # bwd initial h
                whh = [wsb[f"whh{l}f"], wsb[f"whh{l}b"]]

                def fv(tile, elem_off, dims):
                    a = tile[:]
                    return bass.AP(tensor=a.tensor, offset=a.offset + elem_off,
                                   ap=[a.ap[0]] + dims)

                # tgc double buffer: cols [g0 g1 c0 c1 | g0' g1' c0' c1'] so the
                # i*g and f*c products run as ONE DVE op against sg's [i,f] cols
                tgc0 = cst.tile([128, 8], f32, tag="tgc0", name="tgc0")
                tgc1 = cst.tile([128, 8], f32, tag="tgc1", name="tgc1")
                tgc = [tgc0, tgc1]
                nc.gpsimd.memset(tgc[0][:], 0.0)
                nsteps_eff = 0 if variant in ("norec", "nocoll", "nopre") else NSTEPS
                for s in range(nsteps_eff):
                    tA, tB = s, NSTEPS - 1 - s
                    cur, nxt = tgc[s % 2], tgc[(s + 1) % 2]
                    gps = gpool.tile([128, 16], f32, tag="g")
                    # pre-activation values SEED the PSUM tile via ONE
                    # full-tile identity matmul (start=True); the recurrent
                    # matvecs then accumulate on top. start is bank-granular:
                    # two start=True sub-region writes clobber each other, and
                    # wide-accumulate over per-column groups is also broken.
                    jump = (tB - tA) * 16 + 8
                    nc.tensor.matmul(gps[:], ident[:],
                                     fv(pre_t, tA * 16, [[jump, 2], [1, 8]]),
                                     start=True, stop=False)
                    for ci in range(2):
                        rdcol = tA if ci == 0 else tB + 2
                        for j in range(8):
                            for kc in range(2):
                                nc.tensor.matmul(
                                    gps[:, ci * 8 + j:ci * 8 + j + 1],
                                    whh[ci][:, kc, j, :],
                                    hb[:, rdcol, ci * 2 + kc:ci * 2 + kc + 1],
                                    start=False, stop=(kc == 1))
                    sg = scr.tile([128, 12], f32, tag="sg")
                    nc.scalar.activation(sg[:], fv(gps, 0, [[8, 2], [1, 6]]), AF.Sigmoid)
                    nc.scalar.activation(fv(cur, 0, [[4, 2], [1, 2]]),
                                         fv(gps, 6, [[8, 2], [1, 2]]), AF.Tanh)
                    uw = scr.tile([128, 8], f32, tag="uw")
                    nc.vector.tensor_tensor(out=uw[:], in0=fv(sg, 0, [[6, 2], [1, 4]]),
                                            in1=cur[:], op=OP.mult)
                    nc.vector.tensor_tensor(
                        out=fv(nxt, 2, [[4, 2], [1, 2]]),
                        in0=fv(uw, 0, [[4, 2], [1, 2]]),
                        in1=fv(uw, 2, [[4, 2], [1, 2]]), op=OP.add)
                    tc_ = scr.tile([128, 4], f32, tag="tc")
                    nc.scalar.activation(tc_[:], fv(nxt, 2, [[4, 2], [1, 2]]), AF.Tanh)
                    hjump = ((tB + 1) - (tA + 1)) * 4 + 2
                    nc.vector.tensor_tensor(
                        out=fv(hb, (tA + 1) * 4, [[hjump, 2], [1, 2]]),
                        in0=fv(sg, 4, [[6, 2], [1, 2]]), in1=tc_[:], op=OP.mult)

                # ---- export valid H and all-gather ----
                # fwd valid: cols W+1 .. W+SEG ; bwd valid: cols 1 .. SEG
                if export:
                    for di, col0 in enumerate((W + 1, 1)):
                        for bi in range(2):
                            nc.sync.dma_start(hloc[0][di, :, bi, :],
                                              hb[:, col0:col0 + SEG, di * 2 + bi])
                    if variant not in ("nocoll", "nopre"):
                        # gather straight into the padded tensor's middle so
                        # neighbor reads need no separate copy
                        nc.gpsimd.collective_compute(
                            "AllGather", OP.bypass, replica_groups=RG,
                            ins=[hloc[0][:].opt()],
                            outs=[hgat0p[1:NCORES + 1].opt()])
                return hb

            run_layer(0, XT0, 2, 0, W, export=True)

            # ---------- assemble layer-1 input (neighbor segments, dynamic) ----------
            zt = xg_pool.tile([128, 2 * 2 * SEG], f16, tag="zt")
            nc.vector.memset(zt[:], 0.0)
            nc.sync.dma_start(hgat0p[0], zt[:])
            nc.sync.dma_start(hgat0p[NCORES + 1], zt[:])
            pid = nc.partition_id()
            XT1 = xtp.tile([128, 4, 3 * SEG], f16, tag="xt1")
            for si in range(3):
                for di in range(2):
                    for kc in range(2):
                        nc.sync.dma_start(
                            XT1[:, di * 2 + kc, si * SEG:(si + 1) * SEG],
                            hgat0p[bass.ds(pid + si, 1), di, :, kc, :])

            hb1 = run_layer(1, XT1, 4, SEG - W, SEG, export=False)

            gpool_cm.__exit__(None, None, None)
            main_psum.__exit__(None, None, None)

            # ---------- scoring projections (segment-local) ----------
            # s_head / s_dep for this core's SEG tokens, read from hb1 directly:
            # moving operand per (di, kc) is hb1[:, col0:col0+SEG, di*2+kc]
            # (fwd valid cols W+1.., bwd valid cols 1..), matching wm chunk order
            # [hf0, hf1, hb0, hb1].
            sps = ctx.enter_context(tc.tile_pool(name="sps", bufs=2, space="PSUM"))
            sv = xtp.tile([1, 2 * SEG], f32, tag="sv")
            for vi in range(2):  # 0: head, 1: dep
                ps = sps.tile([1, SEG], f32, tag="svps")
                for kcc in range(4):
                    di, kc = kcc // 2, kcc % 2
                    col0 = W + 1 if di == 0 else 1
                    nc.tensor.matmul(ps[:], wm_sb[:, vi * 4 + kcc:vi * 4 + kcc + 1],
                                     hb1[:, col0:col0 + SEG, di * 2 + kc],
                                     start=(kcc == 0), stop=(kcc == 3))
                nc.scalar.activation(sv[0:1, vi * SEG:(vi + 1) * SEG], ps[:], AF.Copy)
            nc.sync.dma_start(svec_d[:], sv[:])

    nc.compile()
    return nc


def _shard_map(f, mesh, in_specs, out_specs):
    import jax
    try:
        return jax.shard_map(f, mesh=mesh, in_specs=in_specs, out_specs=out_specs,
                             check_vma=False)
    except TypeError:
        pass
    try:
        return jax.shard_map(f, mesh=mesh, in_specs=in_specs, out_specs=out_specs,
                             check_rep=False)
    except (TypeError, AttributeError):
        from jax.experimental.shard_map import shard_map as sm
        return sm(f, mesh=mesh, in_specs=in_specs, out_specs=out_specs, check_rep=False)


def _make_exec(nc):
    """Build a cached jitted shard_map dispatch for a compiled Bass program."""
    import jax
    from jax.sharding import Mesh, PartitionSpec, NamedSharding
    from concourse import mybir
    from concourse.bass2jax import (install_neuronx_cc_hook, _bass_exec_p,
                                    partition_id_tensor)

    install_neuronx_cc_hook()

    partition_name = nc.partition_id_tensor.name if nc.partition_id_tensor else None
    in_names, out_names, out_avals = [], [], []
    for alloc in nc.m.functions[0].allocations:
        if not isinstance(alloc, mybir.MemoryLocationSet):
            continue
        name = alloc.memorylocations[0].name
        if alloc.kind == "ExternalInput":
            if name != partition_name:
                in_names.append(name)
        elif alloc.kind == "ExternalOutput":
            out_names.append(name)
            out_avals.append(jax.core.ShapedArray(
                tuple(alloc.tensor_shape), mybir.dt.np(alloc.dtype)))
    all_in_names = in_names + out_names + ([partition_name] if partition_name else [])

    def _body(*args):
        operands = list(args)
        if partition_name is not None:
            operands.append(partition_id_tensor())
        outs = _bass_exec_p.bind(
            *operands, out_avals=tuple(out_avals), in_names=tuple(all_in_names),
            out_names=tuple(out_names), lowering_input_output_aliases=(),
            sim_require_finite=True, sim_require_nnan=True, nc=nc)
        return tuple(outs)

    devices = jax.devices()[:NCORES]
    mesh = Mesh(np.asarray(devices), ("core",))
    spec = PartitionSpec("core")
    n_in, n_out = len(in_names), len(out_names)
    # No donation: the kernel writes every output element, so the zero
    # "output-seed" operands are reusable read-only buffers (a persistent
    # upload), saving a per-call zeros dispatch + donation bookkeeping.
    sharded = jax.jit(
        _shard_map(_body, mesh, (spec,) * (n_in + n_out), (spec,) * n_out),
        keep_unused=True)
    shard8 = NamedSharding(mesh, spec)
    zeros = tuple(
        jax.device_put(np.zeros((NCORES * a.shape[0], *a.shape[1:]), a.dtype), shard8)
        for a in out_avals)

    return dict(sharded=sharded, zeros=zeros, shard8=shard8,
                in_names=in_names, out_avals=out_avals, jax=jax)


def _get_ctx():
    if "exec" in _C:
        return _C
    nc = _build_program()
    _C.update(_make_exec(nc))
    _C.update(nc=nc, dev={}, fp={}, exec=True)
    return _C


def _sample(a):
    return a.reshape(-1)[::997].copy()


def _weights_same(ctx, inputs):
    """Pure check: do the passed weight arrays match what is resident on
    device? Object-identity fast path with a sampled in-place-mutation guard;
    full compare for fresh array objects."""
    fp = ctx["fp"]
    if not fp:
        return False
    for k in _WEIGHT_KEYS:
        a = inputs[k]
        if fp["ref"][k] is a:
            if not np.array_equal(_sample(a), fp["smp"][k]):
                return False
        elif not np.array_equal(fp["cpy"][k], a):
            return False
    return True


def _update_weights(ctx, inputs):
    """Unconditional host prep + device upload of all weight tensors."""
    fp = ctx["fp"]
    fp["ref"] = {k: inputs[k] for k in _WEIGHT_KEYS}
    fp["cpy"] = {k: inputs[k].copy() for k in _WEIGHT_KEYS}
    fp["smp"] = {k: _sample(inputs[k]) for k in _WEIGHT_KEYS}

    base = {}
    for l in (0, 1):
        for d in "fb":
            wih_t, whh_t, bcol = _prep_chain_weights(
                inputs[f"Wih{l}{d}"], inputs[f"Whh{l}{d}"], inputs[f"b{l}{d}"])
            base[f"wih{l}{d}"] = wih_t
            base[f"whh{l}{d}"] = whh_t
            base[f"bcol{l}{d}"] = bcol
    wm = inputs["Wm"].astype(np.float16)
    wm_t = np.zeros((128, 8), np.float16)
    for kc in range(8):
        wm_t[:, kc] = wm[kc * 128:(kc + 1) * 128]
    base["wm"] = wm_t

    jax, shard8 = ctx["jax"], ctx["shard8"]
    dev = ctx["dev"]
    for nm in ctx["in_names"]:
        if nm == "xrow":
            continue
        if nm.startswith("bwarm"):
            l, d = nm[5], nm[6]
            percore = []
            for c in range(NCORES):
                bw = base[f"bcol{l}{d}"].copy()
                if (d == "f" and c == 0) or (d == "b" and c == NCORES - 1):
                    bw[:, 0:6] += FORCE  # force i, f, o gates to zero state
                percore.append(bw)
            arr = np.concatenate(percore, axis=0)
        else:
            arr = np.concatenate([base[nm]] * NCORES, axis=0)
        dev[nm] = jax.device_put(arr, shard8)

    ctx["E16"] = inputs["E"].astype(np.float16)
    ctx["args_tmpl"] = [None if nm == "xrow" else dev[nm] for nm in ctx["in_names"]]
    ctx["xrow_pos"] = ctx["in_names"].index("xrow")


def _prep_out_buf():
    """Allocate the fresh output buffer and pre-fault its pages (one write
    per 4KB page; rows are 8KB = 2 pages). Safe to run before the device
    results arrive — used to hide the fault cost inside the fetch window."""
    buf = np.zeros((T, T), np.float32)
    buf[:, 0] = 0.0
    buf[:, 1024] = 0.0
    return buf


def _finish(s_head, s_dep, bm_val, buf=None):
    """scores = tanh(s_head[i] + s_dep[j] + bm) * triu(k=1), computed only on
    the upper-triangular blocks; the rest of the fresh zeros buffer is never
    touched (calloc pages stay zero)."""
    B = 256
    nb = T // B
    if "ftri" not in _C:
        _C["ftri"] = np.triu(np.ones((B, B), np.float32), k=1)
    tri = _C["ftri"]
    if "fscratch" not in _C:
        _C["fscratch"] = np.empty((B, B), np.float32)
    scratch = _C["fscratch"]
    if buf is None:
        buf = _prep_out_buf()
    sh = (s_head + bm_val).astype(np.float32)
    for bi in range(nb):
        r = slice(bi * B, (bi + 1) * B)
        shb = sh[r][:, None]
        for bj in range(bi, nb):
            c = slice(bj * B, (bj + 1) * B)
            # add into the cache-resident scratch, tanh streams to the output;
            # diagonal blocks pre-mask the INPUT (tanh(0)=0) so the mask
            # multiply stays in cache instead of an in-memory RMW on buf
            np.add(shb, s_dep[c][None, :], out=scratch)
            if bj == bi:
                scratch *= tri
            np.tanh(scratch, out=buf[r, c])
    return buf


def _enqueue_and_fetch(ctx, widx, reuse_xrow=False):
    """Gather embedding rows from the cached E16, enqueue the device program
    with the resident weights, fetch svec. Returns the [NCORES*2, SEG] array.
    With reuse_xrow, the previous call's uploaded xrow is reused (caller has
    verified word_idx is unchanged; E staleness is caught by _weights_same)."""
    jax = ctx["jax"]
    if not (reuse_xrow and "xrow_dev" in ctx):
        xrow = ctx["E16"][widx[_IDX]]                   # [NCORES*SPAN, D] f16
        ctx["xrow_dev"] = jax.device_put(xrow, ctx["shard8"])   # async enqueue
        ctx["widx_cpy"] = widx.copy()
    args = ctx["args_tmpl"]
    args[ctx["xrow_pos"]] = ctx["xrow_dev"]
    return ctx["sharded"](*args, *ctx["zeros"])[0]


def kernel(**inputs):
    import threading
    inputs = {k: np.asarray(v) for k, v in inputs.items()}
    ctx = _get_ctx()
    bm_val = float(np.asarray(inputs["bm"]).reshape(-1)[0])

    t0 = time.time()
    widx = inputs["word_idx"].astype(np.int64)
    buf = None
    if not ctx["fp"]:
        # first call: serial prep + upload, then run
        _update_weights(ctx, inputs)
        sv = np.asarray(_enqueue_and_fetch(ctx, widx))
    else:
        # optimistic: enqueue against the resident weights immediately and
        # overlap the weight-fingerprint validation with the fetch's fixed
        # ~80ms round-trip window; redo properly in the rare mismatch case
        # (note: reusing the previous call's uploaded xrow buffer measured
        # WORSE — a fresh device_put pipelines better through the relay)
        out = _enqueue_and_fetch(ctx, widx)
        box = {}

        def _fetch():
            try:
                box["sv"] = np.asarray(out)
            except Exception as e:       # surfaced after join via refetch
                box["err"] = e
        th = threading.Thread(target=_fetch)
        th.start()
        # both run inside the fetch's ~80ms round-trip window:
        ok = _weights_same(ctx, inputs)
        buf = _prep_out_buf()
        th.join()
        if ok and "sv" in box:
            sv = box["sv"]
        else:
            if not ok:
                _update_weights(ctx, inputs)
            sv = np.asarray(_enqueue_and_fetch(ctx, widx))

    sv = sv.reshape(NCORES, 2, SEG)                     # per-core segment vectors
    s_head = np.ascontiguousarray(sv[:, 0, :]).reshape(T)
    s_dep = np.ascontiguousarray(sv[:, 1, :]).reshape(T)

    scores = _finish(s_head, s_dep, bm_val, buf=buf)
    globals()["LAST_EXEC_WALL_S"] = time.time() - t0
    return scores


# revision 52
# speedup vs baseline: 1.1019x; 1.0218x over previous
"""Trainium2 Bass kernel for nn_DependencyParsingNetwork (2-layer BiLSTM + pair scoring).

Strategy (8 NeuronCores, SPMD single program):
- T=2048 sequence is split into 8 segments of 256, one per core. Each core runs
  its segment of every LSTM chain (layer x direction) with a warmup window of W
  steps before(/after) the segment: LSTM forget gates make the initial-state
  influence decay below fp precision within W steps.
- Boundary cores force-zero their out-of-range warmup via large negative gate
  biases, making segment 0 (and the reversed tail) exact.
- Recurrent matvec: h (fp16) is the stationary PE operand per 128x128 Whh^T
  block; gates accumulate in PSUM fp32, land as [128 partitions x 8 cols] so
  the sigmoid/tanh + cell update run on full-width ACT/DVE ops.
- Cross-core handoff between layers via AllGather collectives (fp16).
- The device returns only s_head/s_dep ([2, T] f32, 16KB); the separable pair
  score tanh(s_head[i] + s_dep[j] + bm) * triu_mask is finished on host (~40ms)
  instead of shipping the 16MB score matrix over the axon tunnel (~400ms).
- Dispatch layer: the Bass program is compiled once and wrapped in a single
  jax.jit(shard_map) that is cached across kernel() calls (run_bass_kernel_spmd
  re-jits per call, which costs seconds). Weights are prepped + uploaded once
  and kept resident on device, revalidated per call by np.array_equal
  fingerprints; only the gathered embedding rows (1.5MB fp16) move per call.
"""

import time
import numpy as np

T = 2048
H = 256
NCORES = 8
SEG = T // NCORES
W = 32                                          # warmup steps (truncation err ~4e-5, fp16 noise ~7e-4)
NSTEPS = SEG + W                                # steps per chain per core
SPAN = SEG + 2 * W                              # input span per core
FORCE = -60.0                                   # gate-forcing bias
V, D = 32000, 256
# gate column order within the 8 j-chunks: [i0 i1 f0 f1 o0 o1 g0 g1]
SRC_BLK = [0, 1, 2, 3, 6, 7, 4, 5]              # source 128-row block in pytorch i,f,g,o order

# per-core gathered input rows (clipped at sequence edges)
_IDX = np.concatenate([
    np.clip(np.arange(c * SEG - W, (c + 1) * SEG + W), 0, T - 1) for c in range(NCORES)
])

_WEIGHT_KEYS = ["E", "Wih0f", "Whh0f", "b0f", "Wih0b", "Whh0b", "b0b",
                "Wih1f", "Whh1f", "b1f", "Wih1b", "Whh1b", "b1b", "Wm"]

_C = {}


def _prep_chain_weights(Wih, Whh, b):
    """Host-side layout prep for one LSTM chain. Returns (wih_t, whh_t, bcol)."""
    KC = Wih.shape[1] // 128
    wih_t = np.zeros((128, KC, 8, 128), np.float16)
    whh_t = np.zeros((128, 2, 8, 128), np.float16)
    bcol = np.zeros((128, 8), np.float32)
    for j in range(8):
        rows = slice(SRC_BLK[j] * 128, (SRC_BLK[j] + 1) * 128)
        for kc in range(KC):
            wih_t[:, kc, j, :] = Wih[rows, kc * 128:(kc + 1) * 128].T.astype(np.float16)
        for kc in range(2):
            whh_t[:, kc, j, :] = Whh[rows, kc * 128:(kc + 1) * 128].T.astype(np.float16)
        bcol[:, j] = b[rows]
    return wih_t, whh_t, bcol


def _build_program(variant="full"):
    import concourse.bacc as bacc
    import concourse.bass as bass
    import concourse.tile as tile
    from concourse import mybir
    from concourse.masks import make_identity

    f32, f16 = mybir.dt.float32, mybir.dt.float16
    AF = mybir.ActivationFunctionType
    OP = mybir.AluOpType

    nc = bacc.Bacc("TRN2", target_bir_lowering=False, debug=False, num_devices=NCORES)

    # ---------------- I/O tensors (per core) ----------------
    ein = lambda name, shape, dt: nc.dram_tensor(name, shape, dt, kind="ExternalInput")
    xrow_d = ein("xrow", [SPAN, D], f16)
    w_in = {}
    for l in (0, 1):
        KC = 2 if l == 0 else 4
        for d in "fb":
            w_in[f"wih{l}{d}"] = ein(f"wih{l}{d}", [128, KC, 8, 128], f16)
            w_in[f"whh{l}{d}"] = ein(f"whh{l}{d}", [128, 2, 8, 128], f16)
            w_in[f"bcol{l}{d}"] = ein(f"bcol{l}{d}", [128, 8], f32)
            w_in[f"bwarm{l}{d}"] = ein(f"bwarm{l}{d}", [128, 8], f32)
    wm_d = ein("wm", [128, 8], f16)          # [k, kc] head chunks 0..3, dep 4..7
    svec_d = nc.dram_tensor("svec", [1, 2 * SEG], f32, kind="ExternalOutput")

    # internal DRAM for the layer-0 collective (layer 1 output stays core-local);
    # gather target is padded by one segment on each side so neighbor reads
    # need no clamping
    hloc = [nc.dram_tensor("h0loc", [2, 128, 2, SEG], f16, kind="Internal")]
    hgat0p = nc.dram_tensor("h0gatp", [NCORES + 2, 2, 128, 2, SEG], f16,
                            kind="Internal", addr_space="Shared")

    RG = [list(range(NCORES))]

    with tile.TileContext(nc) as tc:
        import contextlib
        ctx = contextlib.ExitStack()
        with ctx:
            consts = ctx.enter_context(tc.tile_pool(name="consts", bufs=1))
            xtp = ctx.enter_context(tc.tile_pool(name="xt", bufs=1))
            prep = ctx.enter_context(tc.tile_pool(name="pre", bufs=1))
            hbufp = ctx.enter_context(tc.tile_pool(name="hbuf", bufs=1))
            scr = ctx.enter_context(tc.tile_pool(name="scr", bufs=4))
            cst = ctx.enter_context(tc.tile_pool(name="cst", bufs=3))
            xg_pool = ctx.enter_context(tc.tile_pool(name="xg", bufs=2))

            # ---------- load constants ----------
            wsb = {}
            for k, t_d in w_in.items():
                sh = list(t_d.shape)
                dt = f16 if k.startswith(("wih", "whh")) else f32
                wt = consts.tile(sh, dt, tag=k)
                nc.sync.dma_start(wt[:], t_d[:])
                wsb[k] = wt
            wm_sb = consts.tile([128, 8], f16, tag="wm")
            nc.sync.dma_start(wm_sb[:], wm_d[:])
            ident = consts.tile([128, 128], f16, tag="ident")
            make_identity(nc, ident[:])

            main_psum = tc.tile_pool(name="mainps", bufs=2, space="PSUM")
            gpool = pps = None

            # ---------- embedding gather + XT0 (transpose on PE) ----------
            pps = main_psum.__enter__()
            gpool_cm = tc.tile_pool(name="gps", bufs=2, space="PSUM")
            gpool = gpool_cm.__enter__()

            XT0 = xtp.tile([128, 2, SPAN], f16, tag="xt0")
            off = 0
            while off < SPAN:
                rc = min(128, SPAN - off)
                xg = xg_pool.tile([128, 256], f16, tag="xg")
                nc.sync.dma_start(xg[0:rc, :], xrow_d[off:off + rc, :])
                for kc in range(2):
                    tp = pps.tile([128, 128], f16, tag="tps")
                    nc.tensor.transpose(tp[:, 0:rc], xg[0:rc, kc * 128:(kc + 1) * 128],
                                        ident[0:rc, 0:rc])
                    nc.scalar.activation(XT0[:, kc, off:off + rc], tp[:, 0:rc], AF.Copy)
                off += rc

            # ---------- per-layer pipeline ----------
            def run_layer(l, xt_src, KC, tofs_a, tofs_b, export):
                """xt_src: [128, KC, *] fp16 feature-major input. Returns the hb
                state tile; if export, also DMAs valid H to hloc[0] and
                all-gathers into hgat[0]."""
                pre_t = prep.tile([128, NSTEPS, 16], f16, tag="pre")
                for ci, d in enumerate("fb" if variant != "nopre" else ""):
                    wih = wsb[f"wih{l}{d}"]
                    tofs = tofs_a if ci == 0 else tofs_b
                    for j in range(8):
                        ps = pps.tile([128, NSTEPS], f32, tag="preps")
                        for kc in range(KC):
                            nc.tensor.matmul(ps[:], wih[:, kc, j, :],
                                             xt_src[:, kc, tofs:tofs + NSTEPS],
                                             start=(kc == 0), stop=(kc == KC - 1))
                        # bias add + cast, with gate-forcing bias on the warmup range
                        if ci == 0:
                            wlo, whi = 0, W
                        else:
                            wlo, whi = SEG, NSTEPS
                        bwarm = wsb[f"bwarm{l}{d}"]
                        bcol = wsb[f"bcol{l}{d}"]
                        jc = ci * 8 + j
                        if wlo > 0:
                            nc.scalar.activation(pre_t[:, 0:wlo, jc], ps[:, 0:wlo],
                                                 AF.Identity, bias=bcol[:, j:j + 1])
                        nc.scalar.activation(pre_t[:, wlo:whi, jc], ps[:, wlo:whi],
                                             AF.Identity, bias=bwarm[:, j:j + 1])
                        if whi < NSTEPS:
                            nc.scalar.activation(pre_t[:, whi:NSTEPS, jc], ps[:, whi:NSTEPS],
                                                 AF.Identity, bias=bcol[:, j:j + 1])

                # ---- recurrence (both chains interleaved on this core) ----
                hb = hbufp.tile([128, NSTEPS + 2, 4], f16, tag="hbuf")
                nc.gpsimd.memset(hb[:, 0, 0:2], 0.0)            # fwd initial h
                nc.gpsimd.memset(hb[:, NSTEPS + 1, 2:4], 0.0)   # bwd initial h
                whh = [wsb[f"whh{l}f"], wsb[f"whh{l}b"]]

                def fv(tile, elem_off, dims):
                    a = tile[:]
                    return bass.AP(tensor=a.tensor, offset=a.offset + elem_off,
                                   ap=[a.ap[0]] + dims)

                # tgc double buffer: cols [g0 g1 c0 c1 | g0' g1' c0' c1'] so the
                # i*g and f*c products run as ONE DVE op against sg's [i,f] cols
                tgc0 = cst.tile([128, 8], f32, tag="tgc0", name="tgc0")
                tgc1 = cst.tile([128, 8], f32, tag="tgc1", name="tgc1")
                tgc = [tgc0, tgc1]
                nc.gpsimd.memset(tgc[0][:], 0.0)
                nsteps_eff = 0 if variant in ("norec", "nocoll", "nopre") else NSTEPS
                for s in range(nsteps_eff):
                    tA, tB = s, NSTEPS - 1 - s
                    cur, nxt = tgc[s % 2], tgc[(s + 1) % 2]
                    gps = gpool.tile([128, 16], f32, tag="g")
                    # pre-activation values SEED the PSUM tile via ONE
                    # full-tile identity matmul (start=True); the recurrent
                    # matvecs then accumulate on top. start is bank-granular:
                    # two start=True sub-region writes clobber each other, and
                    # wide-accumulate over per-column groups is also broken.
                    jump = (tB - tA) * 16 + 8
                    nc.tensor.matmul(gps[:], ident[:],
                                     fv(pre_t, tA * 16, [[jump, 2], [1, 8]]),
                                     start=True, stop=False)
                    for ci in range(2):
                        rdcol = tA if ci == 0 else tB + 2
                        for j in range(8):
                            for kc in range(2):
                                nc.tensor.matmul(
                                    gps[:, ci * 8 + j:ci * 8 + j + 1],
                                    whh[ci][:, kc, j, :],
                                    hb[:, rdcol, ci * 2 + kc:ci * 2 + kc + 1],
                                    start=False, stop=(kc == 1))
                    sg = scr.tile([128, 12], f32, tag="sg")
                    nc.scalar.activation(sg[:], fv(gps, 0, [[8, 2], [1, 6]]), AF.Sigmoid)
                    nc.scalar.activation(fv(cur, 0, [[4, 2], [1, 2]]),
                                         fv(gps, 6, [[8, 2], [1, 2]]), AF.Tanh)
                    uw = scr.tile([128, 8], f32, tag="uw")
                    nc.vector.tensor_tensor(out=uw[:], in0=fv(sg, 0, [[6, 2], [1, 4]]),
                                            in1=cur[:], op=OP.mult)
                    nc.vector.tensor_tensor(
                        out=fv(nxt, 2, [[4, 2], [1, 2]]),
                        in0=fv(uw, 0, [[4, 2], [1, 2]]),
                        in1=fv(uw, 2, [[4, 2], [1, 2]]), op=OP.add)
                    tc_ = scr.tile([128, 4], f32, tag="tc")
                    nc.scalar.activation(tc_[:], fv(nxt, 2, [[4, 2], [1, 2]]), AF.Tanh)
                    hjump = ((tB + 1) - (tA + 1)) * 4 + 2
                    nc.vector.tensor_tensor(
                        out=fv(hb, (tA + 1) * 4, [[hjump, 2], [1, 2]]),
                        in0=fv(sg, 4, [[6, 2], [1, 2]]), in1=tc_[:], op=OP.mult)

                # ---- export valid H and all-gather ----
                # fwd valid: cols W+1 .. W+SEG ; bwd valid: cols 1 .. SEG
                if export:
                    for di, col0 in enumerate((W + 1, 1)):
                        for bi in range(2):
                            nc.sync.dma_start(hloc[0][di, :, bi, :],
                                              hb[:, col0:col0 + SEG, di * 2 + bi])
                    if variant not in ("nocoll", "nopre"):
                        # gather straight into the padded tensor's middle so
                        # neighbor reads need no separate copy
                        nc.gpsimd.collective_compute(
                            "AllGather", OP.bypass, replica_groups=RG,
                            ins=[hloc[0][:].opt()],
                            outs=[hgat0p[1:NCORES + 1].opt()])
                return hb

            run_layer(0, XT0, 2, 0, W, export=True)

            # ---------- assemble layer-1 input (neighbor segments, dynamic) ----------
            zt = xg_pool.tile([128, 2 * 2 * SEG], f16, tag="zt")
            nc.vector.memset(zt[:], 0.0)
            nc.sync.dma_start(hgat0p[0], zt[:])
            nc.sync.dma_start(hgat0p[NCORES + 1], zt[:])
            pid = nc.partition_id()
            XT1 = xtp.tile([128, 4, 3 * SEG], f16, tag="xt1")
            for si in range(3):
                for di in range(2):
                    for kc in range(2):
                        nc.sync.dma_start(
                            XT1[:, di * 2 + kc, si * SEG:(si + 1) * SEG],
                            hgat0p[bass.ds(pid + si, 1), di, :, kc, :])

            hb1 = run_layer(1, XT1, 4, SEG - W, SEG, export=False)

            gpool_cm.__exit__(None, None, None)
            main_psum.__exit__(None, None, None)

            # ---------- scoring projections (segment-local) ----------
            # s_head / s_dep for this core's SEG tokens, read from hb1 directly:
            # moving operand per (di, kc) is hb1[:, col0:col0+SEG, di*2+kc]
            # (fwd valid cols W+1.., bwd valid cols 1..), matching wm chunk order
            # [hf0, hf1, hb0, hb1].
            sps = ctx.enter_context(tc.tile_pool(name="sps", bufs=2, space="PSUM"))
            sv = xtp.tile([1, 2 * SEG], f32, tag="sv")
            for vi in range(2):  # 0: head, 1: dep
                ps = sps.tile([1, SEG], f32, tag="svps")
                for kcc in range(4):
                    di, kc = kcc // 2, kcc % 2
                    col0 = W + 1 if di == 0 else 1
                    nc.tensor.matmul(ps[:], wm_sb[:, vi * 4 + kcc:vi * 4 + kcc + 1],
                                     hb1[:, col0:col0 + SEG, di * 2 + kc],
                                     start=(kcc == 0), stop=(kcc == 3))
                nc.scalar.activation(sv[0:1, vi * SEG:(vi + 1) * SEG], ps[:], AF.Copy)
            nc.sync.dma_start(svec_d[:], sv[:])

    nc.compile()
    return nc


def _shard_map(f, mesh, in_specs, out_specs):
    import jax
    try:
        return jax.shard_map(f, mesh=mesh, in_specs=in_specs, out_specs=out_specs,
                             check_vma=False)
    except TypeError:
        pass
    try:
        return jax.shard_map(f, mesh=mesh, in_specs=in_specs, out_specs=out_specs,
                             check_rep=False)
    except (TypeError, AttributeError):
        from jax.experimental.shard_map import shard_map as sm
        return sm(f, mesh=mesh, in_specs=in_specs, out_specs=out_specs, check_rep=False)


def _make_exec(nc):
    """Build a cached jitted shard_map dispatch for a compiled Bass program."""
    import jax
    from jax.sharding import Mesh, PartitionSpec, NamedSharding
    from concourse import mybir
    from concourse.bass2jax import (install_neuronx_cc_hook, _bass_exec_p,
                                    partition_id_tensor)

    install_neuronx_cc_hook()

    partition_name = nc.partition_id_tensor.name if nc.partition_id_tensor else None
    in_names, out_names, out_avals = [], [], []
    for alloc in nc.m.functions[0].allocations:
        if not isinstance(alloc, mybir.MemoryLocationSet):
            continue
        name = alloc.memorylocations[0].name
        if alloc.kind == "ExternalInput":
            if name != partition_name:
                in_names.append(name)
        elif alloc.kind == "ExternalOutput":
            out_names.append(name)
            out_avals.append(jax.core.ShapedArray(
                tuple(alloc.tensor_shape), mybir.dt.np(alloc.dtype)))
    all_in_names = in_names + out_names + ([partition_name] if partition_name else [])

    def _body(*args):
        operands = list(args)
        if partition_name is not None:
            operands.append(partition_id_tensor())
        outs = _bass_exec_p.bind(
            *operands, out_avals=tuple(out_avals), in_names=tuple(all_in_names),
            out_names=tuple(out_names), lowering_input_output_aliases=(),
            sim_require_finite=True, sim_require_nnan=True, nc=nc)
        return tuple(outs)

    devices = jax.devices()[:NCORES]
    mesh = Mesh(np.asarray(devices), ("core",))
    spec = PartitionSpec("core")
    n_in, n_out = len(in_names), len(out_names)
    # No donation: the kernel writes every output element, so the zero
    # "output-seed" operands are reusable read-only buffers (a persistent
    # upload), saving a per-call zeros dispatch + donation bookkeeping.
    sharded = jax.jit(
        _shard_map(_body, mesh, (spec,) * (n_in + n_out), (spec,) * n_out),
        keep_unused=True)
    shard8 = NamedSharding(mesh, spec)
    zeros = tuple(
        jax.device_put(np.zeros((NCORES * a.shape[0], *a.shape[1:]), a.dtype), shard8)
        for a in out_avals)

    return dict(sharded=sharded, zeros=zeros, shard8=shard8,
                in_names=in_names, out_avals=out_avals, jax=jax)


def _get_ctx():
    if "exec" in _C:
        return _C
    nc = _build_program()
    _C.update(_make_exec(nc))
    _C.update(nc=nc, dev={}, fp={}, exec=True)
    return _C


def _sample(a):
    return a.reshape(-1)[::997].copy()


def _weights_same(ctx, inputs):
    """Pure check: do the passed weight arrays match what is resident on
    device? Object-identity fast path with a sampled in-place-mutation guard;
    full compare for fresh array objects."""
    fp = ctx["fp"]
    if not fp:
        return False
    for k in _WEIGHT_KEYS:
        a = inputs[k]
        if fp["ref"][k] is a:
            if not np.array_equal(_sample(a), fp["smp"][k]):
                return False
        elif not np.array_equal(fp["cpy"][k], a):
            return False
    return True


def _update_weights(ctx, inputs):
    """Unconditional host prep + device upload of all weight tensors."""
    fp = ctx["fp"]
    fp["ref"] = {k: inputs[k] for k in _WEIGHT_KEYS}
    fp["cpy"] = {k: inputs[k].copy() for k in _WEIGHT_KEYS}
    fp["smp"] = {k: _sample(inputs[k]) for k in _WEIGHT_KEYS}

    base = {}
    for l in (0, 1):
        for d in "fb":
            wih_t, whh_t, bcol = _prep_chain_weights(
                inputs[f"Wih{l}{d}"], inputs[f"Whh{l}{d}"], inputs[f"b{l}{d}"])
            base[f"wih{l}{d}"] = wih_t
            base[f"whh{l}{d}"] = whh_t
            base[f"bcol{l}{d}"] = bcol
    wm = inputs["Wm"].astype(np.float16)
    wm_t = np.zeros((128, 8), np.float16)
    for kc in range(8):
        wm_t[:, kc] = wm[kc * 128:(kc + 1) * 128]
    base["wm"] = wm_t

    jax, shard8 = ctx["jax"], ctx["shard8"]
    dev = ctx["dev"]
    for nm in ctx["in_names"]:
        if nm == "xrow":
            continue
        if nm.startswith("bwarm"):
            l, d = nm[5], nm[6]
            percore = []
            for c in range(NCORES):
                bw = base[f"bcol{l}{d}"].copy()
                if (d == "f" and c == 0) or (d == "b" and c == NCORES - 1):
                    bw[:, 0:6] += FORCE  # force i, f, o gates to zero state
                percore.append(bw)
            arr = np.concatenate(percore, axis=0)
        else:
            arr = np.concatenate([base[nm]] * NCORES, axis=0)
        dev[nm] = jax.device_put(arr, shard8)

    ctx["E16"] = inputs["E"].astype(np.float16)
    ctx["args_tmpl"] = [None if nm == "xrow" else dev[nm] for nm in ctx["in_names"]]
    ctx["xrow_pos"] = ctx["in_names"].index("xrow")


def _prep_out_buf():
    """Allocate the fresh output buffer and pre-fault its pages (one write
    per 4KB page; rows are 8KB = 2 pages). Safe to run before the device
    results arrive — used to hide the fault cost inside the fetch window."""
    buf = np.zeros((T, T), np.float32)
    buf[:, 0] = 0.0
    buf[:, 1024] = 0.0
    return buf


def _finish(s_head, s_dep, bm_val, buf=None):
    """scores = tanh(s_head[i] + s_dep[j] + bm) * triu(k=1), computed only on
    the upper-triangular blocks; the rest of the fresh zeros buffer is never
    touched (calloc pages stay zero)."""
    B = 256
    nb = T // B
    if "ftri" not in _C:
        _C["ftri"] = np.triu(np.ones((B, B), np.float32), k=1)
    tri = _C["ftri"]
    if "fscratch" not in _C:
        _C["fscratch"] = np.empty((B, B), np.float32)
    scratch = _C["fscratch"]
    if buf is None:
        buf = _prep_out_buf()
    sh = (s_head + bm_val).astype(np.float32)
    for bi in range(nb):
        r = slice(bi * B, (bi + 1) * B)
        shb = sh[r][:, None]
        for bj in range(bi, nb):
            c = slice(bj * B, (bj + 1) * B)
            # add into the cache-resident scratch, tanh streams to the output;
            # diagonal blocks pre-mask the INPUT (tanh(0)=0) so the mask
            # multiply stays in cache instead of an in-memory RMW on buf
            np.add(shb, s_dep[c][None, :], out=scratch)
            if bj == bi:
                scratch *= tri
            np.tanh(scratch, out=buf[r, c])
    return buf


def _enqueue_and_fetch(ctx, widx, reuse_xrow=False):
    """Gather embedding rows from the cached E16, enqueue the device program
    with the resident weights, fetch svec. Returns the [NCORES*2, SEG] array.
    With reuse_xrow, the previous call's uploaded xrow is reused (caller has
    verified word_idx is unchanged; E staleness is caught by _weights_same)."""
    jax = ctx["jax"]
    if not (reuse_xrow and "xrow_dev" in ctx):
        xrow = ctx["E16"][widx[_IDX]]                   # [NCORES*SPAN, D] f16
        ctx["xrow_dev"] = jax.device_put(xrow, ctx["shard8"])   # async enqueue
        ctx["widx_cpy"] = widx.copy()
    args = ctx["args_tmpl"]
    args[ctx["xrow_pos"]] = ctx["xrow_dev"]
    fn = ctx.get("aot")
    if fn is not None:
        return fn(*args, *ctx["zeros"])[0]
    out = ctx["sharded"](*args, *ctx["zeros"])[0]
    try:
        # AOT handle skips per-call pjit dispatch overhead (~0.3ms); the
        # lower+compile is a cache hit after the pjit call above
        ctx["aot"] = ctx["sharded"].lower(*args, *ctx["zeros"]).compile()
    except Exception:
        ctx["aot"] = ctx["sharded"]
    return out


def kernel(**inputs):
    import threading
    inputs = {k: np.asarray(v) for k, v in inputs.items()}
    ctx = _get_ctx()
    bm_val = float(np.asarray(inputs["bm"]).reshape(-1)[0])

    t0 = time.time()
    widx = inputs["word_idx"].astype(np.int64)
    buf = None
    if not ctx["fp"]:
        # first call: serial prep + upload, then run
        _update_weights(ctx, inputs)
        sv = np.asarray(_enqueue_and_fetch(ctx, widx))
    else:
        # optimistic: enqueue against the resident weights immediately and
        # overlap the weight-fingerprint validation with the fetch's fixed
        # ~80ms round-trip window; redo properly in the rare mismatch case
        # (note: reusing the previous call's uploaded xrow buffer measured
        # WORSE — a fresh device_put pipelines better through the relay)
        out = _enqueue_and_fetch(ctx, widx)
        box = {}

        def _fetch():
            try:
                box["sv"] = np.asarray(out)
            except Exception as e:       # surfaced after join via refetch
                box["err"] = e
        th = threading.Thread(target=_fetch)
        th.start()
        # both run inside the fetch's ~80ms round-trip window:
        ok = _weights_same(ctx, inputs)
        buf = _prep_out_buf()
        th.join()
        if ok and "sv" in box:
            sv = box["sv"]
        else:
            if not ok:
                _update_weights(ctx, inputs)
            sv = np.asarray(_enqueue_and_fetch(ctx, widx))

    sv = sv.reshape(NCORES, 2, SEG)                     # per-core segment vectors
    s_head = np.ascontiguousarray(sv[:, 0, :]).reshape(T)
    s_dep = np.ascontiguousarray(sv[:, 1, :]).reshape(T)

    scores = _finish(s_head, s_dep, bm_val, buf=buf)
    globals()["LAST_EXEC_WALL_S"] = time.time() - t0
    return scores
